# revision 21
# baseline (speedup 1.0000x reference)
"""GAT message-passing + h@h.T self-similarity on 8 Trainium2 NeuronCores.

Strategy (graph/data parallel per sharding hint):
  - Nodes padded N=10000 -> NPAD=10240. Dst rows are PERMUTED by descending
    in-degree and the 80 sorted row-tiles are dealt round-robin to the 8
    cores, so every core shares one per-slot neighbor-table width KJ[j] (max
    over its 8-tile group) and the padded edge table shrinks from NPAD*Kmax
    to ~E (+5%).
  - Kernel A (per core):
      phase 1: h_gat[n] = [x@W.T (bf16) | a_src, a_dst (f32)] packed into
        512-byte rows for ALL nodes, computed from a host-transposed bf16 x.T
        resident in SBUF (full-rate bf16 matmuls, no per-tile transposes);
        PSUM->SBUF copies go into an 8-tile staging buffer so DRAM writes are
        one DMA instruction per 8 tiles (the shared HWDGE descriptor
        generator costs ~625ns per DMA instruction, so instruction count
        matters as much as bytes).
      phase 2: per own dst tile: ONE InstDMAGatherAnt (GpSimd mlp library)
        gathers all 128*K neighbor rows (k-major index list, wrapped int16
        layout); the per-dst-row self-loop is forced to slot 0 so the row's
        own a_dst rides along in the gather. Edge softmax along K with a
        host-precomputed additive -1e30 pad mask folded into the score op
        (denominator via the exp's accum_out; output normalized once at the
        end). Unnormalized weighted accumulation split between 2 interleaved
        DVE FMA chains and an Act-built-diag + PE matmul chain accumulating
        in PSUM. The epilogue fuses 1/den, bias, leaky_relu(0.02), residual,
        and a masked per-partition sum-of-squares partial (host sums the
        128 values). All phase-2 tables (indices, mask, x_own|deg|vmask) are
        preloaded in 3 DMA instructions; h rows are staged and written in 2.
  - Host: inverse-permutes h, zero-pads, transposes -> hT [128, NPAD].
  - Kernel B (per core): scale = 1/sum(partials); pred rows = (h_own @ h.T)
    with fp32r matmuls (full PE rate at >=256-wide tiles) reading the
    host-transposed hT; the scale is folded into the PSUM->SBUF output
    copies (alternating DVE/Act) which land in a full row-tile staging
    buffer, written with ONE DMA instruction per 128-row tile; only the
    10000 real columns are written.
"""

import numpy as np
import ml_dtypes

import concourse.bass as bass
import concourse.bacc as bacc
import concourse.mybir as mybir
import concourse.tile as tile
from concourse.bass_utils import run_bass_kernel_spmd
from concourse.masks import make_identity
from concourse import library_config

NC = 8
N = 10000
D = 128
P = 128
NPAD = 10240
RPC = NPAD // NC          # rows per core (1280)
TPC = RPC // P            # row tiles per core (10)
NT = NPAD // P            # total row tiles (80)
ES = 256                  # gather row: 256 bf16 = 512B; [h | a_src | a_dst] bf16 + pad
CHUNK = 64                # K slots per dma_gather instruction
XODW = 132                # xod row: x_own(128) | deg | vmask | 2 pad
F32 = mybir.dt.float32
F32R = mybir.dt.float32r
BF16 = mybir.dt.bfloat16
I32 = mybir.dt.int32
I16 = mybir.dt.int16
AF = mybir.ActivationFunctionType
ALU = mybir.AluOpType


def build_kernel_a(KJ: list[int]) -> bass.Bass:
    KMAX = max(KJ)
    SUMK = sum(KJ)
    nc = bacc.Bacc("TRN2", target_bir_lowering=False)
    xt_in = nc.declare_dram_parameter("xt", [D, NPAD], BF16, isOutput=False)
    # wpack = [W | att_src | att_dst | bias^T]
    wpack_in = nc.declare_dram_parameter("wpack", [D, D + 3], F32, isOutput=False)
    mask_in = nc.declare_dram_parameter("mask", [P, max(SUMK, 1)], F32, isOutput=False)
    idx_in = nc.declare_dram_parameter("idx16", [P, max(8 * SUMK, 1)], I16, isOutput=False)
    xod_in = nc.declare_dram_parameter("xod", [RPC, XODW], F32, isOutput=False)
    hout_out = nc.declare_dram_parameter("hout", [RPC, D], F32, isOutput=True)
    part_out = nc.declare_dram_parameter("partial", [P, 1], F32, isOutput=True)

    hgat = nc.dram_tensor("hgat", [NPAD, ES], BF16)

    with tile.TileContext(nc) as tc:
        with (
            tc.tile_pool(name="const", bufs=1) as cp,
            tc.tile_pool(name="ps_small", bufs=2, space="PSUM") as pp,
        ):
            ident = cp.tile([P, P], F32)
            make_identity(nc, ident[:])
            ones_row = cp.tile([1, P], F32)
            nc.vector.memset(ones_row[:], 1.0)
            identb = cp.tile([P, P], BF16)
            nc.vector.tensor_copy(out=identb[:], in_=ident[:])
            # all gpsimd standard-library work is above; switch to mlp for
            # the InstDMAGatherAnt gathers below (same-engine program order)
            nc.gpsimd.load_library(library_config.mlp)

            wpack = cp.tile([D, D + 3], F32)
            nc.sync.dma_start(out=wpack[:], in_=wpack_in[:, :])
            mask_c = cp.tile([P, max(SUMK, 1)], F32)
            idx16 = cp.tile([P, max(8 * SUMK, 1)], I16)
            xod = cp.tile([P, TPC * XODW], F32)

            wsb = wpack[:, 0:D]
            asrc = wpack[:, D : D + 1]
            adst = wpack[:, D + 1 : D + 2]
            biasT = wpack[:, D + 2 : D + 3]

            # Wext = [W.T | v_src | v_dst] in bf16, with v_* = W.T @ att_*
            wext = cp.tile([D, D + 2], BF16)
            wt_ps = pp.tile([P, P], F32, space="PSUM", tag="tps")
            nc.tensor.transpose(out=wt_ps[:], in_=wsb, identity=ident[:])
            nc.vector.tensor_copy(out=wext[:, 0:D], in_=wt_ps[:])
            vs_ps = pp.tile([P, 1], F32, space="PSUM", tag="vps")
            nc.tensor.matmul(out=vs_ps[:], lhsT=wsb, rhs=asrc, start=True, stop=True)
            nc.vector.tensor_copy(out=wext[:, D : D + 1], in_=vs_ps[:])
            vd_ps = pp.tile([P, 1], F32, space="PSUM", tag="vps")
            nc.tensor.matmul(out=vd_ps[:], lhsT=wsb, rhs=adst, start=True, stop=True)
            nc.vector.tensor_copy(out=wext[:, D + 1 : D + 2], in_=vd_ps[:])

            # bias broadcast to all partitions: biasT -> [1,D] row -> [P,D]
            br_ps = pp.tile([1, P], F32, space="PSUM", tag="vps")
            nc.tensor.matmul(out=br_ps[:], lhsT=biasT, rhs=ident[:], start=True, stop=True)
            bias_row = cp.tile([1, D], F32)
            nc.vector.tensor_copy(out=bias_row[:], in_=br_ps[:])
            b_ps = pp.tile([P, D], F32, space="PSUM", tag="tps")
            nc.tensor.matmul(out=b_ps[:], lhsT=ones_row[:], rhs=bias_row[:], start=True, stop=True)
            bias_b = cp.tile([P, D], F32)
            nc.vector.tensor_copy(out=bias_b[:], in_=b_ps[:])

            # ---- phase 1: h_gat rows for all NPAD nodes ----
            GRP = 4  # tiles per staged DRAM write
            with (
                tc.tile_pool(name="p1", bufs=1) as p1,
                tc.tile_pool(name="p1o", bufs=3) as p1o,
                tc.tile_pool(name="p1ps", bufs=4, space="PSUM") as p1p,
            ):
                xT = p1.tile([D, NPAD], BF16)
                for l in range(4):
                    c0, c1 = l * (NPAD // 4), (l + 1) * (NPAD // 4)
                    nc.sync.dma_start(out=xT[:, c0:c1], in_=xt_in[:, c0:c1])
                nc.sync.dma_start(out=idx16[:], in_=idx_in[:, :])
                nc.sync.dma_start(out=mask_c[:], in_=mask_in[:, :])
                nc.sync.dma_start(
                    out=xod[:].rearrange("p (t c) -> p t c", c=XODW),
                    in_=xod_in[:, :].rearrange("(t p) c -> p t c", p=P),
                )
                copy_engines = [nc.vector, nc.scalar]
                for g in range(NT // GRP):
                    stag = p1o.tile([P, GRP * ES], BF16, tag="stag", name="stag")
                    for i in range(GRP):
                        t = g * GRP + i
                        he_ps = p1p.tile([P, D + 2], F32, space="PSUM", tag="hps", name="he_ps")
                        nc.tensor.matmul(
                            out=he_ps[:],
                            lhsT=xT[:, t * P : (t + 1) * P],
                            rhs=wext[:],
                            start=True,
                            stop=True,
                        )
                        eng = copy_engines[t % len(copy_engines)]
                        if eng is nc.scalar:
                            nc.scalar.activation(
                                out=stag[:, i * ES : i * ES + D + 2], in_=he_ps[:],
                                func=AF.Copy,
                            )
                        else:
                            eng.tensor_copy(
                                out=stag[:, i * ES : i * ES + D + 2], in_=he_ps[:]
                            )
                    nc.sync.dma_start(
                        out=hgat[g * GRP * P : (g + 1) * GRP * P, :].rearrange(
                            "(t p) c -> p t c", p=P
                        ),
                        in_=stag[:].rearrange("p (t c) -> p t c", c=ES),
                    )

            # ---- phase 2: per own dst tile: gather + softmax + aggregate ----
            ss_acc = cp.tile([P, 1], F32)
            nc.vector.memset(ss_acc[:], 0.0)
            hstag = cp.tile([P, TPC * D], F32)

            with (
                tc.tile_pool(name="gather", bufs=3) as gp,
                tc.tile_pool(name="small", bufs=3) as sp,
                tc.tile_pool(name="acc", bufs=2) as ap_,
                tc.tile_pool(name="diag", bufs=4) as dgp,
                tc.tile_pool(name="pe_ps", bufs=2, space="PSUM") as pep,
            ):
                koff = 0
                for t in range(TPC):
                    K = KJ[t]
                    if K == 0:
                        continue
                    xot = xod[:, t * XODW : t * XODW + D]
                    vmt = xod[:, t * XODW + D + 1 : t * XODW + D + 2]

                    # gather the 128*K neighbor rows of h_gat (chunked to the
                    # per-instruction index limit)
                    G = gp.tile([P, K * ES], BF16, tag="G", name="G")
                    Gv = G[:].rearrange("p (k e) -> p k e", e=ES)
                    for c0 in range(0, K, CHUNK):
                        c1 = min(c0 + CHUNK, K)
                        nc.gpsimd.dma_gather(
                            Gv[:, c0:c1],
                            hgat[:, :],
                            idx16[:, 8 * (koff + c0) : 8 * (koff + c1)],
                            P * (c1 - c0), P * (c1 - c0), ES,
                            single_packet=False,
                        )
                    mask_t = mask_c[:, koff : koff + K]
                    koff += K
                    Gf = G[:].bitcast(F32).rearrange("p (k e) -> p k e", e=ES // 2)

                    # scores: leaky_relu(a_src[j] + a_dst[i], 0.2) + (-1e30 on
                    # pad slots, folded in before the exp); self loop is slot 0
                    adst_bf = Gv[:, 0, D + 1 : D + 2]
                    adst_own = sp.tile([P, 1], F32, tag="ado", name="adst_own")
                    nc.vector.tensor_copy(out=adst_own[:], in_=adst_bf)
                    S = sp.tile([P, K], F32, tag="S", name="S")
                    nc.vector.scalar_tensor_tensor(
                        out=S[:], in0=Gv[:, :, D : D + 1], scalar=adst_own[:, 0:1],
                        in1=mask_t, op0=ALU.add, op1=ALU.add,
                    )
                    S2 = sp.tile([P, K], F32, tag="S2", name="S2")
                    nc.vector.scalar_tensor_tensor(
                        out=S2[:], in0=S[:], scalar=0.2, in1=S[:], op0=ALU.mult, op1=ALU.max
                    )

                    # exp with in-instruction denominator accumulation
                    Ex = sp.tile([P, K], F32, tag="Ex", name="Ex")
                    den = sp.tile([P, 1], F32, tag="den", name="den")
                    nc.scalar.activation(out=Ex[:], in_=S2[:], func=AF.Exp, accum_out=den[:])
                    rden = sp.tile([P, 1], F32, tag="rden", name="rden")
                    nc.vector.tensor_scalar_add(out=den[:], in0=den[:], scalar1=1e-16)
                    nc.vector.reciprocal(out=rden[:], in_=den[:])

                    # unnormalized weighted aggregation, split between
                    # 2 interleaved DVE FMA chains and an Act+PE diag-matmul
                    # chain accumulating in PSUM
                    n_pe = min(K, max(0, round(0.37 * K) + 1))
                    kd = list(range(K - n_pe))
                    kp = list(range(K - n_pe, K))
                    accs = []
                    for ci, ks in enumerate((kd[0::2], kd[1::2])):
                        if not ks:
                            continue
                        acc = ap_.tile([P, D], F32, tag=f"acc{ci}", name="acc")
                        accs.append(acc[:])
                        nc.vector.tensor_scalar_mul(
                            out=acc[:], in0=Gv[:, ks[0], 0:D], scalar1=Ex[:, ks[0] : ks[0] + 1]
                        )
                        for k in ks[1:]:
                            nc.vector.scalar_tensor_tensor(
                                out=acc[:], in0=Gv[:, k, 0:D], scalar=Ex[:, k : k + 1],
                                in1=acc[:], op0=ALU.mult, op1=ALU.add,
                            )
                    if kp:
                        accp = pep.tile([P, D], F32, space="PSUM", tag="aps", name="accp")
                        for j, k in enumerate(kp):
                            diag = dgp.tile([P, P], BF16, tag="diag", name="diag")
                            nc.scalar.activation(
                                out=diag[:], in_=identb[:], func=AF.Copy,
                                scale=Ex[:, k : k + 1],
                            )
                            nc.tensor.matmul(
                                out=accp[:], lhsT=diag[:], rhs=Gv[:, k, 0:D],
                                start=(j == 0), stop=(j == len(kp) - 1),
                            )
                        accs.append(accp[:])
                    acc0 = accs[0]
                    for other in accs[1:]:
                        nc.vector.tensor_tensor(
                            out=acc0, in0=acc0, in1=other, op=ALU.add
                        )

                    # h = leaky_relu(acc/den + bias, 0.02) + x_own
                    hpre = ap_.tile([P, D], F32, tag="hpre", name="hpre")
                    nc.vector.scalar_tensor_tensor(
                        out=hpre[:], in0=acc0, scalar=rden[:], in1=bias_b[:],
                        op0=ALU.mult, op1=ALU.add,
                    )
                    lk = ap_.tile([P, D], F32, tag="lk", name="lk")
                    nc.vector.scalar_tensor_tensor(
                        out=lk[:], in0=hpre[:], scalar=0.02, in1=hpre[:],
                        op0=ALU.mult, op1=ALU.max,
                    )
                    nc.vector.tensor_tensor(
                        out=hstag[:, t * D : (t + 1) * D], in0=lk[:], in1=xot, op=ALU.add
                    )

                    # masked partial sum of squares
                    sq = ap_.tile([P, D], F32, tag="sq", name="sq")
                    ssc = sp.tile([P, 1], F32, tag="ssc", name="ssc")
                    nc.scalar.activation(
                        out=sq[:], in_=hstag[:, t * D : (t + 1) * D],
                        func=AF.Square, accum_out=ssc[:],
                    )
                    nc.vector.scalar_tensor_tensor(
                        out=ss_acc[:], in0=ssc[:], scalar=vmt, in1=ss_acc[:],
                        op0=ALU.mult, op1=ALU.add,
                    )

            HF = TPC // 2
            nc.sync.dma_start(
                out=hout_out[0 : HF * P, :].rearrange("(t p) c -> p t c", p=P),
                in_=hstag[:, 0 : HF * D].rearrange("p (t c) -> p t c", c=D),
            )
            nc.sync.dma_start(
                out=hout_out[HF * P :, :].rearrange("(t p) c -> p t c", p=P),
                in_=hstag[:, HF * D :].rearrange("p (t c) -> p t c", c=D),
            )
            # per-partition partials; the host sums the 128 values
            nc.sync.dma_start(out=part_out[:, :], in_=ss_acc[:])

    nc.finalize()
    return nc


def build_kernel_b() -> bass.Bass:
    nc = bacc.Bacc("TRN2", target_bir_lowering=False)
    ht_in = nc.declare_dram_parameter("ht", [D, NPAD], F32R, isOutput=False)
    hto_in = nc.declare_dram_parameter("hto", [D, RPC], F32R, isOutput=False)
    parts_in = nc.declare_dram_parameter("parts", [1, NC], F32, isOutput=False)
    pred_out = nc.declare_dram_parameter("pred", [RPC, N], F32, isOutput=True)

    NB = 512
    NCB = (N + NB - 1) // NB  # 20 column chunks; the last is 272 wide
    NLOAD = 4                 # hT load instructions

    with tile.TileContext(nc) as tc:
        with (
            tc.tile_pool(name="const", bufs=1) as cp,
            tc.tile_pool(name="stg", bufs=2) as sg,
            tc.tile_pool(name="tp_ps", bufs=2, space="PSUM") as tpp,
            tc.tile_pool(name="mm_ps", bufs=6, space="PSUM") as mpp,
        ):
            ones_row = cp.tile([1, P], F32)
            nc.vector.memset(ones_row[:], 1.0)

            # scale = 1 / sum(parts), broadcast to [128,1]
            pt = cp.tile([1, NC], F32)
            nc.sync.dma_start(out=pt[:], in_=parts_in[:, :])
            tot = cp.tile([1, 1], F32)
            nc.vector.tensor_reduce(out=tot[:], in_=pt[:], axis=mybir.AxisListType.X, op=ALU.add)
            rs = cp.tile([1, 1], F32)
            nc.vector.reciprocal(out=rs[:], in_=tot[:])
            sc_ps = tpp.tile([P, 1], F32, space="PSUM", tag="tps")
            nc.tensor.matmul(out=sc_ps[:], lhsT=ones_row[:], rhs=rs[:], start=True, stop=True)
            s_col = cp.tile([P, 1], F32)
            nc.vector.tensor_copy(out=s_col[:], in_=sc_ps[:])

            # h.T resident in SBUF [128 feat x NPAD]
            hTo = cp.tile([P, RPC], F32R)
            nc.sync.dma_start(out=hTo[:], in_=hto_in[:, :])
            hT = cp.tile([P, NPAD], F32R)
            for l in range(NLOAD):
                c0, c1 = l * (NPAD // NLOAD), (l + 1) * (NPAD // NLOAD)
                nc.sync.dma_start(out=hT[:, c0:c1], in_=ht_in[:, c0:c1])

            # 2D-blocked matmul; 1/sum_sq scale folded into the PSUM->SBUF
            # copies (alternating DVE/Act); one staged DMA per 128-row tile
            for rt in range(TPC):
                stag = sg.tile([P, N], F32, tag="stag", name="stag")
                for cb in range(NCB):
                    c0 = cb * NB
                    c1 = min((cb + 1) * NB, N)
                    w = c1 - c0
                    pp_t = mpp.tile([P, NB], F32, space="PSUM", tag="mmps", name="pp_t")
                    nc.tensor.matmul(
                        out=pp_t[:, 0:w],
                        lhsT=hTo[:, rt * P : (rt + 1) * P],
                        rhs=hT[:, c0:c1],
                        start=True,
                        stop=True,
                    )
                    if cb % 2 == 0:
                        nc.vector.tensor_scalar_mul(
                            out=stag[:, c0:c1], in0=pp_t[:, 0:w], scalar1=s_col[:]
                        )
                    else:
                        nc.scalar.activation(
                            out=stag[:, c0:c1], in_=pp_t[:, 0:w], func=AF.Copy,
                            scale=s_col[:],
                        )
                for q in range(4):
                    q0 = (NCB * q // 4) * NB
                    q1 = (NCB * (q + 1) // 4) * NB if q < 3 else N
                    nc.sync.dma_start(
                        out=pred_out[rt * P : (rt + 1) * P, q0:q1],
                        in_=stag[:, q0:q1],
                    )

    nc.finalize()
    return nc


def _prep(x, edge_index, W, att_src, att_dst, bias):
    x = np.asarray(x, dtype=np.float32)
    edge_index = np.asarray(edge_index)
    W = np.asarray(W, dtype=np.float32)
    att_src = np.asarray(att_src, dtype=np.float32).reshape(D, 1)
    att_dst = np.asarray(att_dst, dtype=np.float32).reshape(D, 1)
    bias = np.asarray(bias, dtype=np.float32).reshape(D, 1)

    src = edge_index[0].astype(np.int64)
    dst = edge_index[1].astype(np.int64)

    # CSR by dst (WITHOUT self loops; the self loop is forced into slot 0)
    deg_real = np.bincount(dst, minlength=N)
    eorder = np.argsort(dst, kind="stable")
    src_s = src[eorder].astype(np.int64)
    starts = np.zeros(N + 1, dtype=np.int64)
    starts[1:] = np.cumsum(deg_real)

    deg_pad = np.zeros(NPAD, dtype=np.int64)
    deg_pad[:N] = deg_real + 1  # incl self loop

    # permute rows by descending degree; deal sorted tiles round-robin so all
    # cores share one per-slot width list KJ
    order = np.argsort(-deg_pad, kind="stable")
    KJ = [int(deg_pad[order[(j * NC) * P]]) for j in range(TPC)]
    KMAX = max(KJ)
    SUMK = sum(KJ)

    x_pad = np.zeros((NPAD, D), dtype=np.float32)
    x_pad[:N] = x
    xt_bf16 = np.ascontiguousarray(x_pad.T).astype(ml_dtypes.bfloat16)

    wpack = np.concatenate([W, att_src, att_dst, bias], axis=1).astype(np.float32)

    in_maps = []
    rows_per_core = []
    for c in range(NC):
        tiles = [order[(j * NC + c) * P : (j * NC + c + 1) * P] for j in range(TPC)]
        rows = np.concatenate(tiles)
        rows_per_core.append(rows)
        safe = np.clip(rows, 0, N - 1)
        degs = deg_pad[rows]  # 0 for pad rows

        xod = np.zeros((RPC, XODW), dtype=np.float32)
        xod[:, :D] = x_pad[rows]
        xod[:, D] = degs.astype(np.float32)
        xod[:, D + 1] = (rows < N).astype(np.float32)

        idx16_parts = []
        mask_parts = []
        for j in range(TPC):
            K = KJ[j]
            if K == 0:
                continue
            tr = rows[j * P : (j + 1) * P]
            ts_ = safe[j * P : (j + 1) * P]
            td = degs[j * P : (j + 1) * P]
            idx = np.zeros((P, K), dtype=np.int64)
            idx[:, 0] = np.where(tr < N, tr, 0)
            allcols = np.arange(K, dtype=np.int64)[None, :]
            mask_parts.append(
                np.where(allcols < td[:, None], 0.0, -1e30).astype(np.float32)
            )
            if K > 1:
                cols = allcols[:, 1:]
                valid = cols < td[:, None]
                ptr = np.minimum(starts[ts_][:, None] + (cols - 1), len(src_s) - 1)
                idx[:, 1:] = np.where(valid, src_s[ptr], 0)
            # per-CHUNK k-major lists m = k*128 + p, wrapped [16, 8*ck],
            # replicated to 128 partitions
            for c0 in range(0, K, CHUNK):
                c1 = min(c0 + CHUNK, K)
                m_list = idx[:, c0:c1].T.ravel()
                wrapped = m_list.reshape(8 * (c1 - c0), 16).T
                idx16_parts.append(np.tile(wrapped, (8, 1)).astype(np.int16))
        idx16 = (
            np.concatenate(idx16_parts, axis=1)
            if idx16_parts
            else np.zeros((P, 1), np.int16)
        )
        if idx16.shape[1] < max(8 * SUMK, 1):
            pad = np.zeros((P, max(8 * SUMK, 1) - idx16.shape[1]), np.int16)
            idx16 = np.concatenate([idx16, pad], axis=1)
        mask = (
            np.concatenate(mask_parts, axis=1)
            if mask_parts
            else np.zeros((P, 1), np.float32)
        )
        if mask.shape[1] < max(SUMK, 1):
            mask = np.concatenate(
                [mask, np.zeros((P, max(SUMK, 1) - mask.shape[1]), np.float32)], axis=1
            )

        in_maps.append(
            {
                "xt": xt_bf16,
                "wpack": wpack,
                "mask": mask,
                "idx16": idx16,
                "xod": xod,
            }
        )

    return in_maps, rows_per_core, KJ, x_pad


def kernel(x, edge_index, W, att_src, att_dst, bias, _trace=False, _sim_only=False):
    in_maps_a, rows_per_core, KJ, x_pad = _prep(
        x, edge_index, W, att_src, att_dst, bias
    )

    nc_a = build_kernel_a(KJ)
    nc_b = build_kernel_b()
    if _sim_only:
        kernel.last_modules = (nc_a, nc_b)
        return None

    res_a = run_bass_kernel_spmd(nc_a, in_maps_a, list(range(NC)), trace=_trace)
    ra = res_a.results

    h_nat = np.zeros((NPAD, D), dtype=np.float32)
    for c in range(NC):
        h_nat[rows_per_core[c]] = ra[c]["hout"]
    h_nat[N:] = 0.0
    hT = np.ascontiguousarray(h_nat.T)
    parts = np.array(
        [[float(ra[c]["partial"].sum()) for c in range(NC)]], dtype=np.float32
    )

    in_maps_b = []
    for c in range(NC):
        r0, r1 = c * RPC, (c + 1) * RPC
        in_maps_b.append(
            {"ht": hT, "hto": np.ascontiguousarray(hT[:, r0:r1]), "parts": parts}
        )
    res_b = run_bass_kernel_spmd(nc_b, in_maps_b, list(range(NC)), trace=_trace)
    rb = res_b.results

    pred = np.empty((N, N), dtype=np.float32)
    for c in range(NC):
        r0 = c * RPC
        r1 = min(r0 + RPC, N)
        if r1 > r0:
            pred[r0:r1] = rb[c]["pred"][: r1 - r0, :]

    kernel.last_results = (res_a, res_b)
    kernel.last_modules = (nc_a, nc_b)
    return pred


# revision 22
# speedup vs baseline: 1.0248x; 1.0248x over previous
"""GAT message-passing + h@h.T self-similarity on 8 Trainium2 NeuronCores.

Strategy (graph/data parallel per sharding hint):
  - Nodes padded N=10000 -> NPAD=10240. Dst rows are PERMUTED by descending
    in-degree and the 80 sorted row-tiles are dealt round-robin to the 8
    cores, so every core shares one per-slot neighbor-table width KJ[j] (max
    over its 8-tile group) and the padded edge table shrinks from NPAD*Kmax
    to ~E (+5%).
  - Kernel A (per core):
      phase 1: h_gat[n] = [x@W.T (bf16) | a_src, a_dst (f32)] packed into
        512-byte rows for ALL nodes, computed from a host-transposed bf16 x.T
        resident in SBUF (full-rate bf16 matmuls, no per-tile transposes);
        PSUM->SBUF copies go into an 8-tile staging buffer so DRAM writes are
        one DMA instruction per 8 tiles (the shared HWDGE descriptor
        generator costs ~625ns per DMA instruction, so instruction count
        matters as much as bytes).
      phase 2: per own dst tile: ONE InstDMAGatherAnt (GpSimd mlp library)
        gathers all 128*K neighbor rows (k-major index list, wrapped int16
        layout); the per-dst-row self-loop is forced to slot 0 so the row's
        own a_dst rides along in the gather. Edge softmax along K with a
        host-precomputed additive -1e30 pad mask folded into the score op
        (denominator via the exp's accum_out; output normalized once at the
        end). Unnormalized weighted accumulation split between 2 interleaved
        DVE FMA chains and an Act-built-diag + PE matmul chain accumulating
        in PSUM. The epilogue fuses 1/den, bias, leaky_relu(0.02), residual,
        and a masked per-partition sum-of-squares partial (host sums the
        128 values). All phase-2 tables (indices, mask, x_own|deg|vmask) are
        preloaded in 3 DMA instructions; h rows are staged and written in 2.
  - Host: inverse-permutes h, zero-pads, transposes -> hT [128, NPAD].
  - Kernel B (per core): scale = 1/sum(partials); pred rows = (h_own @ h.T)
    with fp32r matmuls (full PE rate at >=256-wide tiles) reading the
    host-transposed hT; the scale is folded into the PSUM->SBUF output
    copies (alternating DVE/Act) which land in a full row-tile staging
    buffer, written with ONE DMA instruction per 128-row tile; only the
    10000 real columns are written.
"""

import numpy as np
import ml_dtypes

import concourse.bass as bass
import concourse.bacc as bacc
import concourse.mybir as mybir
import concourse.tile as tile
from concourse.bass_utils import run_bass_kernel_spmd
from concourse.masks import make_identity
from concourse import library_config

NC = 8
N = 10000
D = 128
P = 128
NPAD = 10240
RPC = NPAD // NC          # rows per core (1280)
TPC = RPC // P            # row tiles per core (10)
NT = NPAD // P            # total row tiles (80)
ES = 256                  # gather row: 256 bf16 = 512B; [h | a_src | a_dst] bf16 + pad
CHUNK = 64                # K slots per dma_gather instruction
XODW = 132                # xod row: x_own(128) | deg | vmask | 2 pad
F32 = mybir.dt.float32
F32R = mybir.dt.float32r
BF16 = mybir.dt.bfloat16
I32 = mybir.dt.int32
I16 = mybir.dt.int16
AF = mybir.ActivationFunctionType
ALU = mybir.AluOpType


def build_kernel_a(KJ: list[int]) -> bass.Bass:
    KMAX = max(KJ)
    SUMK = sum(KJ)
    nc = bacc.Bacc("TRN2", target_bir_lowering=False)
    xt_in = nc.declare_dram_parameter("xt", [D, NPAD], BF16, isOutput=False)
    # wpack = [W | att_src | att_dst | bias^T]
    wpack_in = nc.declare_dram_parameter("wpack", [D, D + 3], F32, isOutput=False)
    mask_in = nc.declare_dram_parameter("mask", [P, max(SUMK, 1)], F32, isOutput=False)
    idx_in = nc.declare_dram_parameter("idx16", [P, max(8 * SUMK, 1)], I16, isOutput=False)
    xod_in = nc.declare_dram_parameter("xod", [RPC, XODW], F32, isOutput=False)
    hout_out = nc.declare_dram_parameter("hout", [RPC, D], F32, isOutput=True)
    part_out = nc.declare_dram_parameter("partial", [P, 1], F32, isOutput=True)

    hgat = nc.dram_tensor("hgat", [NPAD, ES], BF16)

    with tile.TileContext(nc) as tc:
        with (
            tc.tile_pool(name="const", bufs=1) as cp,
            tc.tile_pool(name="ps_small", bufs=2, space="PSUM") as pp,
        ):
            ident = cp.tile([P, P], F32)
            make_identity(nc, ident[:])
            ones_row = cp.tile([1, P], F32)
            nc.vector.memset(ones_row[:], 1.0)
            identb = cp.tile([P, P], BF16)
            nc.vector.tensor_copy(out=identb[:], in_=ident[:])
            # all gpsimd standard-library work is above; switch to mlp for
            # the InstDMAGatherAnt gathers below (same-engine program order)
            nc.gpsimd.load_library(library_config.mlp)

            wpack = cp.tile([D, D + 3], F32)
            nc.sync.dma_start(out=wpack[:], in_=wpack_in[:, :])
            mask_c = cp.tile([P, max(SUMK, 1)], F32)
            idx16 = cp.tile([P, max(8 * SUMK, 1)], I16)
            xod = cp.tile([P, TPC * XODW], F32)

            wsb = wpack[:, 0:D]
            asrc = wpack[:, D : D + 1]
            adst = wpack[:, D + 1 : D + 2]
            biasT = wpack[:, D + 2 : D + 3]

            # Wext = [W.T | v_src | v_dst] in bf16, with v_* = W.T @ att_*
            wext = cp.tile([D, D + 2], BF16)
            wt_ps = pp.tile([P, P], F32, space="PSUM", tag="tps")
            nc.tensor.transpose(out=wt_ps[:], in_=wsb, identity=ident[:])
            nc.vector.tensor_copy(out=wext[:, 0:D], in_=wt_ps[:])
            vs_ps = pp.tile([P, 1], F32, space="PSUM", tag="vps")
            nc.tensor.matmul(out=vs_ps[:], lhsT=wsb, rhs=asrc, start=True, stop=True)
            nc.vector.tensor_copy(out=wext[:, D : D + 1], in_=vs_ps[:])
            vd_ps = pp.tile([P, 1], F32, space="PSUM", tag="vps")
            nc.tensor.matmul(out=vd_ps[:], lhsT=wsb, rhs=adst, start=True, stop=True)
            nc.vector.tensor_copy(out=wext[:, D + 1 : D + 2], in_=vd_ps[:])

            # bias broadcast to all partitions: biasT -> [1,D] row -> [P,D]
            br_ps = pp.tile([1, P], F32, space="PSUM", tag="vps")
            nc.tensor.matmul(out=br_ps[:], lhsT=biasT, rhs=ident[:], start=True, stop=True)
            bias_row = cp.tile([1, D], F32)
            nc.vector.tensor_copy(out=bias_row[:], in_=br_ps[:])
            b_ps = pp.tile([P, D], F32, space="PSUM", tag="tps")
            nc.tensor.matmul(out=b_ps[:], lhsT=ones_row[:], rhs=bias_row[:], start=True, stop=True)
            bias_b = cp.tile([P, D], F32)
            nc.vector.tensor_copy(out=bias_b[:], in_=b_ps[:])

            # ---- phase 1: h_gat rows for all NPAD nodes ----
            GRP = 8  # tiles per staged DRAM write
            with (
                tc.tile_pool(name="p1", bufs=1) as p1,
                tc.tile_pool(name="p1o", bufs=3) as p1o,
                tc.tile_pool(name="p1ps", bufs=4, space="PSUM") as p1p,
            ):
                xT = p1.tile([D, NPAD], BF16)
                for l in range(4):
                    c0, c1 = l * (NPAD // 4), (l + 1) * (NPAD // 4)
                    nc.sync.dma_start(out=xT[:, c0:c1], in_=xt_in[:, c0:c1])
                nc.sync.dma_start(out=idx16[:], in_=idx_in[:, :])
                nc.sync.dma_start(out=mask_c[:], in_=mask_in[:, :])
                nc.sync.dma_start(
                    out=xod[:].rearrange("p (t c) -> p t c", c=XODW),
                    in_=xod_in[:, :].rearrange("(t p) c -> p t c", p=P),
                )
                copy_engines = [nc.vector, nc.scalar]
                for g in range(NT // GRP):
                    stag = p1o.tile([P, GRP * ES], BF16, tag="stag", name="stag")
                    for i in range(GRP):
                        t = g * GRP + i
                        he_ps = p1p.tile([P, D + 2], F32, space="PSUM", tag="hps", name="he_ps")
                        nc.tensor.matmul(
                            out=he_ps[:],
                            lhsT=xT[:, t * P : (t + 1) * P],
                            rhs=wext[:],
                            start=True,
                            stop=True,
                        )
                        eng = copy_engines[t % len(copy_engines)]
                        if eng is nc.scalar:
                            nc.scalar.activation(
                                out=stag[:, i * ES : i * ES + D + 2], in_=he_ps[:],
                                func=AF.Copy,
                            )
                        else:
                            eng.tensor_copy(
                                out=stag[:, i * ES : i * ES + D + 2], in_=he_ps[:]
                            )
                    nc.sync.dma_start(
                        out=hgat[g * GRP * P : (g + 1) * GRP * P, :].rearrange(
                            "(t p) c -> p t c", p=P
                        ),
                        in_=stag[:].rearrange("p (t c) -> p t c", c=ES),
                    )

            # ---- phase 2: per own dst tile: gather + softmax + aggregate ----
            ss_acc = cp.tile([P, 1], F32)
            nc.vector.memset(ss_acc[:], 0.0)
            hstag = cp.tile([P, TPC * D], F32)

            with (
                tc.tile_pool(name="gather", bufs=4) as gp,
                tc.tile_pool(name="small", bufs=3) as sp,
                tc.tile_pool(name="acc", bufs=2) as ap_,
                tc.tile_pool(name="diag", bufs=4) as dgp,
                tc.tile_pool(name="pe_ps", bufs=2, space="PSUM") as pep,
            ):
                koff = 0
                for t in range(TPC):
                    K = KJ[t]
                    if K == 0:
                        continue
                    xot = xod[:, t * XODW : t * XODW + D]
                    vmt = xod[:, t * XODW + D + 1 : t * XODW + D + 2]

                    # gather the 128*K neighbor rows of h_gat (chunked to the
                    # per-instruction index limit)
                    G = gp.tile([P, K * ES], BF16, tag="G", name="G")
                    Gv = G[:].rearrange("p (k e) -> p k e", e=ES)
                    for c0 in range(0, K, CHUNK):
                        c1 = min(c0 + CHUNK, K)
                        nc.gpsimd.dma_gather(
                            Gv[:, c0:c1],
                            hgat[:, :],
                            idx16[:, 8 * (koff + c0) : 8 * (koff + c1)],
                            P * (c1 - c0), P * (c1 - c0), ES,
                            single_packet=False,
                        )
                    mask_t = mask_c[:, koff : koff + K]
                    koff += K
                    Gf = G[:].bitcast(F32).rearrange("p (k e) -> p k e", e=ES // 2)

                    # scores: leaky_relu(a_src[j] + a_dst[i], 0.2) + (-1e30 on
                    # pad slots, folded in before the exp); self loop is slot 0
                    adst_bf = Gv[:, 0, D + 1 : D + 2]
                    adst_own = sp.tile([P, 1], F32, tag="ado", name="adst_own")
                    nc.vector.tensor_copy(out=adst_own[:], in_=adst_bf)
                    S = sp.tile([P, K], F32, tag="S", name="S")
                    nc.vector.scalar_tensor_tensor(
                        out=S[:], in0=Gv[:, :, D : D + 1], scalar=adst_own[:, 0:1],
                        in1=mask_t, op0=ALU.add, op1=ALU.add,
                    )
                    S2 = sp.tile([P, K], F32, tag="S2", name="S2")
                    nc.vector.scalar_tensor_tensor(
                        out=S2[:], in0=S[:], scalar=0.2, in1=S[:], op0=ALU.mult, op1=ALU.max
                    )

                    # exp with in-instruction denominator accumulation
                    Ex = sp.tile([P, K], F32, tag="Ex", name="Ex")
                    den = sp.tile([P, 1], F32, tag="den", name="den")
                    nc.scalar.activation(out=Ex[:], in_=S2[:], func=AF.Exp, accum_out=den[:])
                    rden = sp.tile([P, 1], F32, tag="rden", name="rden")
                    nc.vector.tensor_scalar_add(out=den[:], in0=den[:], scalar1=1e-16)
                    nc.vector.reciprocal(out=rden[:], in_=den[:])

                    # unnormalized weighted aggregation, split between
                    # 2 interleaved DVE FMA chains and an Act+PE diag-matmul
                    # chain accumulating in PSUM
                    n_pe = min(K, max(0, round(0.37 * K) + 1))
                    kd = list(range(K - n_pe))
                    kp = list(range(K - n_pe, K))
                    accs = []
                    for ci, ks in enumerate((kd[0::2], kd[1::2])):
                        if not ks:
                            continue
                        acc = ap_.tile([P, D], F32, tag=f"acc{ci}", name="acc")
                        accs.append(acc[:])
                        nc.vector.tensor_scalar_mul(
                            out=acc[:], in0=Gv[:, ks[0], 0:D], scalar1=Ex[:, ks[0] : ks[0] + 1]
                        )
                        for k in ks[1:]:
                            nc.vector.scalar_tensor_tensor(
                                out=acc[:], in0=Gv[:, k, 0:D], scalar=Ex[:, k : k + 1],
                                in1=acc[:], op0=ALU.mult, op1=ALU.add,
                            )
                    if kp:
                        accp = pep.tile([P, D], F32, space="PSUM", tag="aps", name="accp")
                        for j, k in enumerate(kp):
                            diag = dgp.tile([P, P], BF16, tag="diag", name="diag")
                            nc.scalar.activation(
                                out=diag[:], in_=identb[:], func=AF.Copy,
                                scale=Ex[:, k : k + 1],
                            )
                            nc.tensor.matmul(
                                out=accp[:], lhsT=diag[:], rhs=Gv[:, k, 0:D],
                                start=(j == 0), stop=(j == len(kp) - 1),
                            )
                        accs.append(accp[:])
                    acc0 = accs[0]
                    for other in accs[1:]:
                        nc.vector.tensor_tensor(
                            out=acc0, in0=acc0, in1=other, op=ALU.add
                        )

                    # h = leaky_relu(acc/den + bias, 0.02) + x_own
                    hpre = ap_.tile([P, D], F32, tag="hpre", name="hpre")
                    nc.vector.scalar_tensor_tensor(
                        out=hpre[:], in0=acc0, scalar=rden[:], in1=bias_b[:],
                        op0=ALU.mult, op1=ALU.add,
                    )
                    lk = ap_.tile([P, D], F32, tag="lk", name="lk")
                    nc.vector.scalar_tensor_tensor(
                        out=lk[:], in0=hpre[:], scalar=0.02, in1=hpre[:],
                        op0=ALU.mult, op1=ALU.max,
                    )
                    nc.vector.tensor_tensor(
                        out=hstag[:, t * D : (t + 1) * D], in0=lk[:], in1=xot, op=ALU.add
                    )

                    # masked partial sum of squares
                    sq = ap_.tile([P, D], F32, tag="sq", name="sq")
                    ssc = sp.tile([P, 1], F32, tag="ssc", name="ssc")
                    nc.scalar.activation(
                        out=sq[:], in_=hstag[:, t * D : (t + 1) * D],
                        func=AF.Square, accum_out=ssc[:],
                    )
                    nc.vector.scalar_tensor_tensor(
                        out=ss_acc[:], in0=ssc[:], scalar=vmt, in1=ss_acc[:],
                        op0=ALU.mult, op1=ALU.add,
                    )

            HF = TPC // 2
            nc.sync.dma_start(
                out=hout_out[0 : HF * P, :].rearrange("(t p) c -> p t c", p=P),
                in_=hstag[:, 0 : HF * D].rearrange("p (t c) -> p t c", c=D),
            )
            nc.sync.dma_start(
                out=hout_out[HF * P :, :].rearrange("(t p) c -> p t c", p=P),
                in_=hstag[:, HF * D :].rearrange("p (t c) -> p t c", c=D),
            )
            # per-partition partials; the host sums the 128 values
            nc.sync.dma_start(out=part_out[:, :], in_=ss_acc[:])

    nc.finalize()
    return nc


def build_kernel_b() -> bass.Bass:
    nc = bacc.Bacc("TRN2", target_bir_lowering=False)
    ht_in = nc.declare_dram_parameter("ht", [D, NPAD], F32R, isOutput=False)
    hto_in = nc.declare_dram_parameter("hto", [D, RPC], F32R, isOutput=False)
    parts_in = nc.declare_dram_parameter("parts", [1, NC], F32, isOutput=False)
    pred_out = nc.declare_dram_parameter("pred", [RPC, N], F32, isOutput=True)

    NB = 512
    NCB = (N + NB - 1) // NB  # 20 column chunks; the last is 272 wide
    NLOAD = 4                 # hT load instructions

    with tile.TileContext(nc) as tc:
        with (
            tc.tile_pool(name="const", bufs=1) as cp,
            tc.tile_pool(name="stg", bufs=2) as sg,
            tc.tile_pool(name="tp_ps", bufs=2, space="PSUM") as tpp,
            tc.tile_pool(name="mm_ps", bufs=6, space="PSUM") as mpp,
        ):
            ones_row = cp.tile([1, P], F32)
            nc.vector.memset(ones_row[:], 1.0)

            # scale = 1 / sum(parts), broadcast to [128,1]
            pt = cp.tile([1, NC], F32)
            nc.sync.dma_start(out=pt[:], in_=parts_in[:, :])
            tot = cp.tile([1, 1], F32)
            nc.vector.tensor_reduce(out=tot[:], in_=pt[:], axis=mybir.AxisListType.X, op=ALU.add)
            rs = cp.tile([1, 1], F32)
            nc.vector.reciprocal(out=rs[:], in_=tot[:])
            sc_ps = tpp.tile([P, 1], F32, space="PSUM", tag="tps")
            nc.tensor.matmul(out=sc_ps[:], lhsT=ones_row[:], rhs=rs[:], start=True, stop=True)
            s_col = cp.tile([P, 1], F32)
            nc.vector.tensor_copy(out=s_col[:], in_=sc_ps[:])

            # h.T resident in SBUF [128 feat x NPAD]
            hTo = cp.tile([P, RPC], F32R)
            nc.sync.dma_start(out=hTo[:], in_=hto_in[:, :])
            hT = cp.tile([P, NPAD], F32R)
            for l in range(NLOAD):
                c0, c1 = l * (NPAD // NLOAD), (l + 1) * (NPAD // NLOAD)
                nc.sync.dma_start(out=hT[:, c0:c1], in_=ht_in[:, c0:c1])

            # 2D-blocked matmul; 1/sum_sq scale folded into the PSUM->SBUF
            # copies (alternating DVE/Act); one staged DMA per 128-row tile
            for rt in range(TPC):
                stag = sg.tile([P, N], F32, tag="stag", name="stag")
                for cb in range(NCB):
                    c0 = cb * NB
                    c1 = min((cb + 1) * NB, N)
                    w = c1 - c0
                    pp_t = mpp.tile([P, NB], F32, space="PSUM", tag="mmps", name="pp_t")
                    nc.tensor.matmul(
                        out=pp_t[:, 0:w],
                        lhsT=hTo[:, rt * P : (rt + 1) * P],
                        rhs=hT[:, c0:c1],
                        start=True,
                        stop=True,
                    )
                    if cb % 2 == 0:
                        nc.vector.tensor_scalar_mul(
                            out=stag[:, c0:c1], in0=pp_t[:, 0:w], scalar1=s_col[:]
                        )
                    else:
                        nc.scalar.activation(
                            out=stag[:, c0:c1], in_=pp_t[:, 0:w], func=AF.Copy,
                            scale=s_col[:],
                        )
                for q in range(4):
                    q0 = (NCB * q // 4) * NB
                    q1 = (NCB * (q + 1) // 4) * NB if q < 3 else N
                    nc.sync.dma_start(
                        out=pred_out[rt * P : (rt + 1) * P, q0:q1],
                        in_=stag[:, q0:q1],
                    )

    nc.finalize()
    return nc


def _prep(x, edge_index, W, att_src, att_dst, bias):
    x = np.asarray(x, dtype=np.float32)
    edge_index = np.asarray(edge_index)
    W = np.asarray(W, dtype=np.float32)
    att_src = np.asarray(att_src, dtype=np.float32).reshape(D, 1)
    att_dst = np.asarray(att_dst, dtype=np.float32).reshape(D, 1)
    bias = np.asarray(bias, dtype=np.float32).reshape(D, 1)

    src = edge_index[0].astype(np.int64)
    dst = edge_index[1].astype(np.int64)

    # CSR by dst (WITHOUT self loops; the self loop is forced into slot 0)
    deg_real = np.bincount(dst, minlength=N)
    eorder = np.argsort(dst, kind="stable")
    src_s = src[eorder].astype(np.int64)
    starts = np.zeros(N + 1, dtype=np.int64)
    starts[1:] = np.cumsum(deg_real)

    deg_pad = np.zeros(NPAD, dtype=np.int64)
    deg_pad[:N] = deg_real + 1  # incl self loop

    # permute rows by descending degree; deal sorted tiles round-robin so all
    # cores share one per-slot width list KJ
    order = np.argsort(-deg_pad, kind="stable")
    KJ = [int(deg_pad[order[(j * NC) * P]]) for j in range(TPC)]
    KMAX = max(KJ)
    SUMK = sum(KJ)

    x_pad = np.zeros((NPAD, D), dtype=np.float32)
    x_pad[:N] = x
    xt_bf16 = np.ascontiguousarray(x_pad.T).astype(ml_dtypes.bfloat16)

    wpack = np.concatenate([W, att_src, att_dst, bias], axis=1).astype(np.float32)

    in_maps = []
    rows_per_core = []
    for c in range(NC):
        tiles = [order[(j * NC + c) * P : (j * NC + c + 1) * P] for j in range(TPC)]
        rows = np.concatenate(tiles)
        rows_per_core.append(rows)
        safe = np.clip(rows, 0, N - 1)
        degs = deg_pad[rows]  # 0 for pad rows

        xod = np.zeros((RPC, XODW), dtype=np.float32)
        xod[:, :D] = x_pad[rows]
        xod[:, D] = degs.astype(np.float32)
        xod[:, D + 1] = (rows < N).astype(np.float32)

        idx16_parts = []
        mask_parts = []
        for j in range(TPC):
            K = KJ[j]
            if K == 0:
                continue
            tr = rows[j * P : (j + 1) * P]
            ts_ = safe[j * P : (j + 1) * P]
            td = degs[j * P : (j + 1) * P]
            idx = np.zeros((P, K), dtype=np.int64)
            idx[:, 0] = np.where(tr < N, tr, 0)
            allcols = np.arange(K, dtype=np.int64)[None, :]
            mask_parts.append(
                np.where(allcols < td[:, None], 0.0, -1e30).astype(np.float32)
            )
            if K > 1:
                cols = allcols[:, 1:]
                valid = cols < td[:, None]
                ptr = np.minimum(starts[ts_][:, None] + (cols - 1), len(src_s) - 1)
                idx[:, 1:] = np.where(valid, src_s[ptr], 0)
            # per-CHUNK k-major lists m = k*128 + p, wrapped [16, 8*ck],
            # replicated to 128 partitions
            for c0 in range(0, K, CHUNK):
                c1 = min(c0 + CHUNK, K)
                m_list = idx[:, c0:c1].T.ravel()
                wrapped = m_list.reshape(8 * (c1 - c0), 16).T
                idx16_parts.append(np.tile(wrapped, (8, 1)).astype(np.int16))
        idx16 = (
            np.concatenate(idx16_parts, axis=1)
            if idx16_parts
            else np.zeros((P, 1), np.int16)
        )
        if idx16.shape[1] < max(8 * SUMK, 1):
            pad = np.zeros((P, max(8 * SUMK, 1) - idx16.shape[1]), np.int16)
            idx16 = np.concatenate([idx16, pad], axis=1)
        mask = (
            np.concatenate(mask_parts, axis=1)
            if mask_parts
            else np.zeros((P, 1), np.float32)
        )
        if mask.shape[1] < max(SUMK, 1):
            mask = np.concatenate(
                [mask, np.zeros((P, max(SUMK, 1) - mask.shape[1]), np.float32)], axis=1
            )

        in_maps.append(
            {
                "xt": xt_bf16,
                "wpack": wpack,
                "mask": mask,
                "idx16": idx16,
                "xod": xod,
            }
        )

    return in_maps, rows_per_core, KJ, x_pad


def kernel(x, edge_index, W, att_src, att_dst, bias, _trace=False, _sim_only=False):
    in_maps_a, rows_per_core, KJ, x_pad = _prep(
        x, edge_index, W, att_src, att_dst, bias
    )

    nc_a = build_kernel_a(KJ)
    nc_b = build_kernel_b()
    if _sim_only:
        kernel.last_modules = (nc_a, nc_b)
        return None

    res_a = run_bass_kernel_spmd(nc_a, in_maps_a, list(range(NC)), trace=_trace)
    ra = res_a.results

    h_nat = np.zeros((NPAD, D), dtype=np.float32)
    for c in range(NC):
        h_nat[rows_per_core[c]] = ra[c]["hout"]
    h_nat[N:] = 0.0
    hT = np.ascontiguousarray(h_nat.T)
    parts = np.array(
        [[float(ra[c]["partial"].sum()) for c in range(NC)]], dtype=np.float32
    )

    in_maps_b = []
    for c in range(NC):
        r0, r1 = c * RPC, (c + 1) * RPC
        in_maps_b.append(
            {"ht": hT, "hto": np.ascontiguousarray(hT[:, r0:r1]), "parts": parts}
        )
    res_b = run_bass_kernel_spmd(nc_b, in_maps_b, list(range(NC)), trace=_trace)
    rb = res_b.results

    pred = np.empty((N, N), dtype=np.float32)
    for c in range(NC):
        r0 = c * RPC
        r1 = min(r0 + RPC, N)
        if r1 > r0:
            pred[r0:r1] = rb[c]["pred"][: r1 - r0, :]

    kernel.last_results = (res_a, res_b)
    kernel.last_modules = (nc_a, nc_b)
    return pred
